# revision 37
# baseline (speedup 1.0000x reference)
"""Trainium2 Bass kernel for segment-wise self-attention pooling.

Computation (per segment s of a ragged atom array):
  logits = A @ Wa ; e = exp(logits) (softmax shift-invariant, logits are O(+-5))
  att    = e / segsum(e) ; pooled[s] = sum_{i in s} att_i * A_i
  out[s] = pooled[s] @ Wo + bo

Strategy (8 NeuronCores, SPMD — one program, per-core data), v2:
  - Shard atoms by contiguous segment ranges, balanced by atom count.
  - Tiles are grouped into RUNS (program-static tile ranges). Within a run,
    every tile's segments fit a per-core 128-segment window [W_run, W_run+128)
    (host-validated). Per-atom window column d_p = seg(p) - W_run is shipped
    as data ([128,1] f32 per tile).
  - Per tile: logits on DVE (mul+reduce vs replicated Wa) or on PE
    (f32r transposes + flipped matmul), batched ACT exp, then ONE op builds
    the full attention matrix E_wide[p,j] = (iota[j]==d_p)*e_p from a
    constant iota tile (no mask DMA). PE matmul E_wide.T @ [A|1|0] (float32r,
    1 cyc/row, base 0) accumulates straight into the run's PSUM window.
  - Run flush: one ACT copy PSUM->SBUF per run (~3/core). Final merge maps
    window rows to global segment rows with small host-built 0/1 matrices
    (data) via a few fp32 matmuls; then reciprocal, scale, PE transpose,
    project with Wo, +bo via K=1 ones matmul, DMA out. Host concatenates.
"""

import sys
import numpy as np

sys.path.insert(0, "/opt/trn_rl_repo")

P = 128
TB = 8            # tiles per DMA block
MAXS = 8          # max segments one tile may touch (validated)
WMAX = 120        # window capacity used during run construction
NCORES = 8
USE_F32R = True
PE_LOGITS_MOD8 = 0   # tiles with (t%8) < this use the PE logits path


def _imports():
    import concourse.bass as bass
    import concourse.bacc as bacc
    import concourse.mybir as mybir
    import concourse.tile as tile
    return bass, bacc, mybir, tile


def plan(index_list, n_atoms, n_cores=NCORES):
    off = np.asarray(index_list, dtype=np.int64)
    S = len(off) - 1
    targets = (np.arange(1, n_cores) * n_atoms) // n_cores
    cuts = np.searchsorted(off, targets)
    for i, t in enumerate(targets):
        c = cuts[i]
        if c > 0 and abs(off[c - 1] - t) < abs(off[c] - t):
            cuts[i] = c - 1
    seg_cuts = np.concatenate([[0], cuts, [S]])
    cores = []
    for c in range(n_cores):
        s0, s1 = int(seg_cuts[c]), int(seg_cuts[c + 1])
        a0, a1 = int(off[s0]), int(off[s1])
        cores.append(dict(s0=s0, s1=s1, a0=a0, a1=a1,
                          n_seg=s1 - s0, n_atom=a1 - a0))
    t_real = [int(np.ceil(cc["n_atom"] / P)) for cc in cores]
    T_pad = int(np.ceil(max(t_real) / TB) * TB)

    for cc in cores:
        a0, a1, s0 = cc["a0"], cc["a1"], cc["s0"]
        seg_atom = np.searchsorted(off, np.arange(a0, a1), side="right") - 1 - s0
        cc["seg_atom"] = seg_atom
        first_seg = np.full(T_pad, -1, dtype=np.int64)
        last_seg = np.full(T_pad, -1, dtype=np.int64)
        for t in range(T_pad):
            lo, hi = t * P, min((t + 1) * P, a1 - a0)
            if lo >= hi:
                continue
            first_seg[t] = seg_atom[lo]
            last_seg[t] = seg_atom[hi - 1]
        cc["first_seg"] = first_seg
        cc["last_seg"] = last_seg
        ok = first_seg >= 0
        assert (last_seg[ok] - first_seg[ok]).max() + 1 <= MAXS

    # ---- runs: maximal tile ranges where every core's segs fit one window
    runs = []
    start = 0
    while start < T_pad:
        end = start + 1
        while end < T_pad:
            fits = True
            for cc in cores:
                fs = cc["first_seg"][start:end + 1]
                ls = cc["last_seg"][start:end + 1]
                m = fs >= 0
                if m.any() and ls[m].max() - fs[m].min() + 1 > WMAX:
                    fits = False
                    break
            if not fits:
                break
            end += 1
        runs.append((start, end))
        start = end
    n_runs = len(runs)

    # per-core per-run window base; merge structure
    n_acc = max(int(np.ceil(cc["n_seg"] / P)) for cc in cores)
    merge_pairs = []   # program-static (run, acc_tile)
    for r, (t0, t1) in enumerate(runs):
        accs = set()
        for cc in cores:
            fs = cc["first_seg"][t0:t1]
            m = fs >= 0
            if not m.any():
                cc.setdefault("W_run", []).append(0)
                continue
            w = int(fs[m].min())
            hi = int(cc["last_seg"][t0:t1][m].max())
            assert hi - w < P
            cc.setdefault("W_run", []).append(w)
            accs.add(min(w // P, n_acc - 1))
            if (w + P - 1) // P != w // P:
                a2 = min((w + P - 1) // P, n_acc - 1)
                accs.add(a2)
            if hi // P != w // P:
                accs.add(min(hi // P, n_acc - 1))
        for a in sorted(accs):
            merge_pairs.append((r, a))
    return dict(S=S, off=off, cores=cores, T_pad=T_pad, runs=runs,
                n_runs=n_runs, n_acc=n_acc, merge_pairs=merge_pairs,
                t_real=t_real)


def build_core_inputs(pl, atom_features, Wa, Wo, bo):
    A = np.asarray(atom_features, dtype=np.float32)
    Wa = np.asarray(Wa, dtype=np.float32).reshape(-1)
    Wo = np.asarray(Wo, dtype=np.float32)
    bo = np.asarray(bo, dtype=np.float32).reshape(1, -1)
    D = A.shape[1]
    Dp = D + 2
    T_pad, runs, n_runs = pl["T_pad"], pl["runs"], pl["n_runs"]
    merge_pairs, n_acc = pl["merge_pairs"], pl["n_acc"]

    wa_rep = np.zeros((P, Dp), dtype=np.float32)
    wa_rep[:, :D] = Wa[None, :]
    wa_cols = np.stack([Wa[:P], Wa[P:]], axis=1).astype(np.float32)  # [128,2]
    wo_packed = np.concatenate([Wo[:P, :], Wo[P:, :]], axis=1)
    ident = np.eye(P, dtype=np.float32)
    iota = np.tile(np.arange(P, dtype=np.float32)[None, :], (P, 1))

    run_of_tile = np.zeros(T_pad, dtype=np.int64)
    for r, (t0, t1) in enumerate(runs):
        run_of_tile[t0:t1] = r

    in_maps = []
    for cc in pl["cores"]:
        a0, a1 = cc["a0"], cc["a1"]
        na = a1 - a0
        a_aug = np.zeros((T_pad * P, Dp), dtype=np.float32)
        a_aug[:na, :D] = A[a0:a1]
        a_aug[:na, D] = 1.0

        dcols = np.full((T_pad, P), -1.0, dtype=np.float32)
        seg_atom = cc["seg_atom"]
        for t in range(T_pad):
            lo, hi = t * P, min((t + 1) * P, na)
            if lo >= hi:
                continue
            w = cc["W_run"][run_of_tile[t]]
            d = seg_atom[lo:hi] - w
            assert d.min() >= 0 and d.max() < P
            dcols[t, :hi - lo] = d.astype(np.float32)
        dcols_packed = (dcols.reshape(T_pad // TB, TB, P)
                        .transpose(0, 2, 1).copy())   # [blocks, 128, TB]

        # merge matrices: window row w of run r -> acc row (W_run + w) - 128*a
        mm = np.zeros((len(merge_pairs), P, P), dtype=np.float32)
        for k, (r, a) in enumerate(merge_pairs):
            wbase = cc["W_run"][r]
            t0, t1 = runs[r]
            fs = cc["first_seg"][t0:t1]
            if not (fs >= 0).any():
                continue
            hi = int(cc["last_seg"][t0:t1][fs >= 0].max())
            for wrow in range(min(P, hi - wbase + 1)):
                g = wbase + wrow
                if a * P <= g < (a + 1) * P:
                    mm[k, wrow, g - a * P] = 1.0

        in_maps.append({
            "a_aug": a_aug, "dcols": dcols_packed, "mergem": mm,
            "wa_rep": wa_rep, "wa_cols": wa_cols,
            "wo_packed": wo_packed, "bo_row": bo,
            "ident": ident, "identr": ident, "iota": iota,
        })
    return in_maps


def build_program(pl, D=256, use_f32r=USE_F32R):
    bass, bacc, mybir, tile = _imports()
    from contextlib import ExitStack
    f32 = mybir.dt.float32
    fmm = mybir.dt.float32r if use_f32r else f32
    Dp = D + 2
    T_pad, runs, n_runs = pl["T_pad"], pl["runs"], pl["n_runs"]
    merge_pairs, n_acc = pl["merge_pairs"], pl["n_acc"]
    n_mm = len(merge_pairs)

    run_of_tile = {}
    for r, (t0, t1) in enumerate(runs):
        for t in range(t0, t1):
            run_of_tile[t] = r

    nc = bacc.Bacc("TRN2", target_bir_lowering=False, debug=False,
                   num_devices=NCORES)
    a_aug_d = nc.dram_tensor("a_aug", [T_pad * P, Dp], fmm, kind="ExternalInput")
    dcols_d = nc.dram_tensor("dcols", [T_pad // TB, P, TB], f32, kind="ExternalInput")
    mergem_d = nc.dram_tensor("mergem", [n_mm, P, P], f32, kind="ExternalInput")
    wa_rep_d = nc.dram_tensor("wa_rep", [P, Dp], fmm, kind="ExternalInput")
    wa_cols_d = nc.dram_tensor("wa_cols", [P, 2], fmm, kind="ExternalInput")
    wo_d = nc.dram_tensor("wo_packed", [P, 2 * D], f32, kind="ExternalInput")
    bo_d = nc.dram_tensor("bo_row", [1, D], f32, kind="ExternalInput")
    ident_d = nc.dram_tensor("ident", [P, P], f32, kind="ExternalInput")
    identr_d = nc.dram_tensor("identr", [P, P], fmm, kind="ExternalInput")
    iota_d = nc.dram_tensor("iota", [P, P], f32, kind="ExternalInput")
    out_d = nc.dram_tensor("out", [n_acc * P, D], f32, kind="ExternalOutput")
    scr_d = nc.dram_tensor("lscratch", [T_pad // 4, 4 * P], f32)

    with ExitStack() as top:
        tc = top.enter_context(tile.TileContext(nc))
        consts = top.enter_context(tc.tile_pool(name="consts", bufs=1))
        wa_rep = consts.tile([P, Dp], fmm)
        nc.sync.dma_start(wa_rep[:], wa_rep_d[:])
        wa_cols = consts.tile([P, 2], fmm)
        nc.sync.dma_start(wa_cols[:], wa_cols_d[:])
        wo_sb = consts.tile([P, 2 * D], f32)
        nc.sync.dma_start(wo_sb[:], wo_d[:])
        bo_sb = consts.tile([1, D], f32)
        nc.sync.dma_start(bo_sb[:], bo_d[:])
        ident = consts.tile([P, P], f32)
        nc.sync.dma_start(ident[:], ident_d[:])
        identr = consts.tile([P, P], fmm)
        nc.sync.dma_start(identr[:], identr_d[:])
        iota = consts.tile([P, P], f32)
        nc.sync.dma_start(iota[:], iota_d[:])
        msb = consts.tile([P, n_mm * P], f32)
        nc.sync.dma_start(
            msb[:].rearrange("p (k m) -> p k m", k=n_mm),
            mergem_d[:].rearrange("k p m -> p k m"))
        ones1 = consts.tile([1, P], f32)
        nc.gpsimd.memset(ones1[:], 1.0)
        flush = consts.tile([P, n_runs * Dp], f32)

        with ExitStack() as loop:
            accp = loop.enter_context(tc.tile_pool(name="accr", bufs=2, space="PSUM"))
            apool = loop.enter_context(tc.tile_pool(name="a", bufs=3))
            dpool2 = loop.enter_context(tc.tile_pool(name="dc", bufs=3))
            cpool = loop.enter_context(tc.tile_pool(name="cols", bufs=3))
            epool = loop.enter_context(tc.tile_pool(name="ecols", bufs=3))
            dpool = loop.enter_context(tc.tile_pool(name="dump", bufs=3))
            Epool = loop.enter_context(tc.tile_pool(name="Ew", bufs=4))
            tpp2 = loop.enter_context(tc.tile_pool(name="tpx", bufs=4, space="PSUM"))
            lrowp = loop.enter_context(tc.tile_pool(name="lrow", bufs=2, space="PSUM"))
            atp = loop.enter_context(tc.tile_pool(name="at", bufs=4))
            erp = loop.enter_context(tc.tile_pool(name="erow", bufs=3))

            a_view = a_aug_d[:].rearrange("(b t p) f -> b p t f", t=TB, p=P)
            state = {"accr": None}
            pending = []

            def emit_emm(pb):
                b, a_st, d_st, ecols = pb
                for tl in range(TB):
                    t = b * TB + tl
                    r = run_of_tile[t]
                    t0, t1 = runs[r]
                    if t == t0:
                        state["accr"] = accp.tile([P, Dp], f32, name="accr")
                    accr = state["accr"]
                    Ew = Epool.tile([P, P], fmm, name="Ew")
                    nc.vector.tensor_scalar(
                        Ew[:], iota[:], d_st[:, tl:tl + 1],
                        ecols[:, tl:tl + 1],
                        op0=mybir.AluOpType.is_equal,
                        op1=mybir.AluOpType.mult)
                    nc.tensor.matmul(accr[:], lhsT=Ew[:],
                                     rhs=a_st[:, tl * Dp:(tl + 1) * Dp],
                                     start=(t == t0), stop=(t == t1 - 1))
                    if t == t1 - 1:
                        nc.scalar.copy(flush[:, r * Dp:(r + 1) * Dp], accr[:])

            for b in range(T_pad // TB):
                a_st = apool.tile([P, TB * Dp], fmm)
                nc.sync.dma_start(a_st[:].rearrange("p (t f) -> p t f", t=TB),
                                  a_view[b])
                import os as _os
                d_st = dpool2.tile([P, TB], f32)
                nc.sync.dma_start(d_st[:], dcols_d[b])
                colbuf = cpool.tile([P, TB], f32)
                pe_block = (b % 3) < int(_os.environ.get("FL_PEB", "2"))
                if not pe_block:
                    for tl in range(TB):
                        dump = dpool.tile([P, Dp], f32)
                        nc.vector.tensor_mul(
                            dump[:], a_st[:, tl * Dp:(tl + 1) * Dp], wa_rep[:])
                        nc.vector.reduce_sum(
                            colbuf[:, tl:tl + 1], dump[:],
                            axis=mybir.AxisListType.X)
                for g in range(TB // 4 if pe_block else 0):
                    tpA = tpp2.tile([P, 4 * P], f32, name="tpA", tag="tp")
                    tpB = tpp2.tile([P, 4 * P], f32, name="tpB", tag="tp")
                    for gi in range(4):
                        tl = g * 4 + gi
                        base = tl * Dp
                        nc.tensor.transpose(
                            tpA[:, gi * P:(gi + 1) * P],
                            a_st[:, base:base + P].bitcast(f32), ident[:])
                        nc.tensor.transpose(
                            tpB[:, gi * P:(gi + 1) * P],
                            a_st[:, base + P:base + 2 * P].bitcast(f32), ident[:])
                    atA = atp.tile([P, 4 * P], fmm, name="atA", tag="at")
                    nc.vector.tensor_copy(atA[:], tpA[:])
                    atB = atp.tile([P, 4 * P], fmm, name="atB", tag="at")
                    nc.scalar.copy(atB[:], tpB[:])
                    lrow = lrowp.tile([1, 4 * P], f32)
                    nc.tensor.matmul(lrow[:], lhsT=wa_cols[:, 0:1], rhs=atA[:],
                                     start=True, stop=False)
                    nc.tensor.matmul(lrow[:], lhsT=wa_cols[:, 1:2], rhs=atB[:],
                                     start=False, stop=True)
                    erow = erp.tile([1, 4 * P], f32)
                    nc.scalar.copy(erow[:], lrow[:])
                    # reshape [1, 4*128] row -> [128, 4] cols via DRAM bounce
                    gg = b * (TB // 4) + g
                    nc.sync.dma_start(scr_d[gg:gg + 1, :], erow[:])
                    nc.sync.dma_start(
                        colbuf[:, g * 4:(g + 1) * 4],
                        scr_d[gg:gg + 1, :].rearrange("o (t p) -> (o p) t", p=P))
                ecols = epool.tile([P, TB], f32)
                nc.scalar.activation(ecols[:], colbuf[:],
                                     mybir.ActivationFunctionType.Exp)
                for tl in range(TB):
                    import os as _os
                    t = b * TB + tl
                    r = run_of_tile[t]
                    t0, t1 = runs[r]
                    if t == t0 and _os.environ.get("FL_EMM", "1") == "1":
                        accr = accp.tile([P, Dp], f32, name="accr")
                    Ew = Epool.tile([P, P], fmm)
                    eng = nc.vector
                    eng.tensor_scalar(
                        Ew[:], iota[:], d_st[:, tl:tl + 1],
                        ecols[:, tl:tl + 1],
                        op0=mybir.AluOpType.is_equal,
                        op1=mybir.AluOpType.mult)
                    if _os.environ.get("FL_EMM", "1") == "1":
                        nc.tensor.matmul(accr[:], lhsT=Ew[:],
                                         rhs=a_st[:, tl * Dp:(tl + 1) * Dp],
                                         start=(t == t0), stop=(t == t1 - 1))
                        if t == t1 - 1:
                            nc.scalar.copy(flush[:, r * Dp:(r + 1) * Dp], accr[:])

        # ---- merge windows into global accumulators
        with ExitStack() as tail:
            acc2p = tail.enter_context(tc.tile_pool(name="acc2", bufs=1, space="PSUM"))
            acc2 = [acc2p.tile([P, Dp], f32, name=f"acc2_{i}")
                    for i in range(n_acc)]
            first = {a: True for a in range(n_acc)}
            last_k = {}
            for k, (r, a) in enumerate(merge_pairs):
                last_k[a] = k
            import os as _os
            if _os.environ.get("FL_EMM", "1") == "1" and _os.environ.get("FL_MERGE", "1") == "1":
                for k, (r, a) in enumerate(merge_pairs):
                    nc.tensor.matmul(acc2[a][:],
                                     lhsT=msb[:, k * P:(k + 1) * P],
                                     rhs=flush[:, r * Dp:(r + 1) * Dp],
                                     start=first[a], stop=(last_k[a] == k))
                    first[a] = False
            else:
                for a in range(n_acc):
                    nc.tensor.matmul(acc2[a][:], lhsT=ones1[:],
                                     rhs=flush[0:1, 0:Dp],
                                     start=True, stop=True)

            spool = tail.enter_context(tc.tile_pool(name="small", bufs=3 * n_acc))
            tpp = tail.enter_context(tc.tile_pool(name="tp", bufs=2, space="PSUM"))
            ptsb = [consts.tile([P, n_acc * P], f32, name=f"ptsb{h}")
                    for h in range(2)]
            for a in range(n_acc):
                dsafe = spool.tile([P, 1], f32)
                nc.vector.tensor_scalar_max(dsafe[:], acc2[a][:, D:D + 1], 1e-30)
                recip = spool.tile([P, 1], f32)
                nc.vector.reciprocal(recip[:], dsafe[:])
                pooledN = spool.tile([P, D], f32)
                nc.vector.tensor_scalar_mul(pooledN[:], acc2[a][:, 0:D], recip[:])
                for h in range(2):
                    tp = tpp.tile([P, P], f32)
                    nc.tensor.transpose(tp[:], pooledN[:, h * P:(h + 1) * P],
                                        ident[:])
                    nc.scalar.copy(ptsb[h][:, a * P:(a + 1) * P], tp[:])
            opp = tail.enter_context(tc.tile_pool(name="outp", bufs=2, space="PSUM"))
            opool = tail.enter_context(tc.tile_pool(name="osb", bufs=2))
            for a in range(n_acc):
                outp = opp.tile([P, D], f32)
                for h in range(2):
                    nc.tensor.matmul(outp[:], lhsT=ptsb[h][:, a * P:(a + 1) * P],
                                     rhs=wo_sb[:, h * D:(h + 1) * D],
                                     start=(h == 0), stop=False)
                nc.tensor.matmul(outp[:], lhsT=ones1[:], rhs=bo_sb[:],
                                 start=False, stop=True)
                outsb = opool.tile([P, D], f32)
                nc.vector.tensor_copy(outsb[:], outp[:])
                nc.sync.dma_start(out_d[a * P:(a + 1) * P, :], outsb[:])
    nc.compile()
    return nc


def kernel(atom_features, index_list, Wa, ba, Wo, bo):
    atom_features = np.asarray(atom_features, dtype=np.float32)
    index_list = np.asarray(index_list)
    n_atoms, D = atom_features.shape
    pl = plan(index_list, n_atoms)
    in_maps = build_core_inputs(pl, atom_features, Wa, Wo, bo)
    nc = build_program(pl, D=D, use_f32r=USE_F32R)

    from concourse.bass_utils import run_bass_kernel_spmd
    res = run_bass_kernel_spmd(nc, in_maps, list(range(NCORES)))
    outs = []
    for c, cc in enumerate(pl["cores"]):
        outs.append(res.results[c]["out"][:cc["n_seg"]])
    return np.concatenate(outs, axis=0)


# revision 38
# speedup vs baseline: 1.0005x; 1.0005x over previous
"""Trainium2 Bass kernel for segment-wise self-attention pooling.

Computation (per segment s of a ragged atom array):
  logits = A @ Wa ; e = exp(logits) (softmax shift-invariant, logits are O(+-5))
  att    = e / segsum(e) ; pooled[s] = sum_{i in s} att_i * A_i
  out[s] = pooled[s] @ Wo + bo

Strategy (8 NeuronCores, SPMD — one program, per-core data), v2:
  - Shard atoms by contiguous segment ranges, balanced by atom count.
  - Tiles are grouped into RUNS (program-static tile ranges). Within a run,
    every tile's segments fit a per-core 128-segment window [W_run, W_run+128)
    (host-validated). Per-atom window column d_p = seg(p) - W_run is shipped
    as data ([128,1] f32 per tile).
  - Per tile: logits on DVE (mul+reduce vs replicated Wa) or on PE
    (f32r transposes + flipped matmul), batched ACT exp, then ONE op builds
    the full attention matrix E_wide[p,j] = (iota[j]==d_p)*e_p from a
    constant iota tile (no mask DMA). PE matmul E_wide.T @ [A|1|0] (float32r,
    1 cyc/row, base 0) accumulates straight into the run's PSUM window.
  - Run flush: one ACT copy PSUM->SBUF per run (~3/core). Final merge maps
    window rows to global segment rows with small host-built 0/1 matrices
    (data) via a few fp32 matmuls; then reciprocal, scale, PE transpose,
    project with Wo, +bo via K=1 ones matmul, DMA out. Host concatenates.
"""

import sys
import numpy as np

sys.path.insert(0, "/opt/trn_rl_repo")

P = 128
TB = 8            # tiles per DMA block
MAXS = 8          # max segments one tile may touch (validated)
WMAX = 120        # window capacity used during run construction
NCORES = 8
USE_F32R = True
PE_LOGITS_MOD8 = 0   # tiles with (t%8) < this use the PE logits path


def _imports():
    import concourse.bass as bass
    import concourse.bacc as bacc
    import concourse.mybir as mybir
    import concourse.tile as tile
    return bass, bacc, mybir, tile


def plan(index_list, n_atoms, n_cores=NCORES):
    off = np.asarray(index_list, dtype=np.int64)
    S = len(off) - 1
    targets = (np.arange(1, n_cores) * n_atoms) // n_cores
    cuts = np.searchsorted(off, targets)
    for i, t in enumerate(targets):
        c = cuts[i]
        if c > 0 and abs(off[c - 1] - t) < abs(off[c] - t):
            cuts[i] = c - 1
    seg_cuts = np.concatenate([[0], cuts, [S]])
    cores = []
    for c in range(n_cores):
        s0, s1 = int(seg_cuts[c]), int(seg_cuts[c + 1])
        a0, a1 = int(off[s0]), int(off[s1])
        cores.append(dict(s0=s0, s1=s1, a0=a0, a1=a1,
                          n_seg=s1 - s0, n_atom=a1 - a0))
    t_real = [int(np.ceil(cc["n_atom"] / P)) for cc in cores]
    T_pad = int(np.ceil(max(t_real) / TB) * TB)

    for cc in cores:
        a0, a1, s0 = cc["a0"], cc["a1"], cc["s0"]
        seg_atom = np.searchsorted(off, np.arange(a0, a1), side="right") - 1 - s0
        cc["seg_atom"] = seg_atom
        first_seg = np.full(T_pad, -1, dtype=np.int64)
        last_seg = np.full(T_pad, -1, dtype=np.int64)
        for t in range(T_pad):
            lo, hi = t * P, min((t + 1) * P, a1 - a0)
            if lo >= hi:
                continue
            first_seg[t] = seg_atom[lo]
            last_seg[t] = seg_atom[hi - 1]
        cc["first_seg"] = first_seg
        cc["last_seg"] = last_seg
        ok = first_seg >= 0
        assert (last_seg[ok] - first_seg[ok]).max() + 1 <= MAXS

    # ---- runs: maximal tile ranges where every core's segs fit one window
    runs = []
    start = 0
    while start < T_pad:
        end = start + 1
        while end < T_pad:
            fits = True
            for cc in cores:
                fs = cc["first_seg"][start:end + 1]
                ls = cc["last_seg"][start:end + 1]
                m = fs >= 0
                if m.any() and ls[m].max() - fs[m].min() + 1 > WMAX:
                    fits = False
                    break
            if not fits:
                break
            end += 1
        runs.append((start, end))
        start = end
    n_runs = len(runs)

    # per-core per-run window base; merge structure
    n_acc = max(int(np.ceil(cc["n_seg"] / P)) for cc in cores)
    merge_pairs = []   # program-static (run, acc_tile)
    for r, (t0, t1) in enumerate(runs):
        accs = set()
        for cc in cores:
            fs = cc["first_seg"][t0:t1]
            m = fs >= 0
            if not m.any():
                cc.setdefault("W_run", []).append(0)
                continue
            w = int(fs[m].min())
            hi = int(cc["last_seg"][t0:t1][m].max())
            assert hi - w < P
            cc.setdefault("W_run", []).append(w)
            accs.add(min(w // P, n_acc - 1))
            if (w + P - 1) // P != w // P:
                a2 = min((w + P - 1) // P, n_acc - 1)
                accs.add(a2)
            if hi // P != w // P:
                accs.add(min(hi // P, n_acc - 1))
        for a in sorted(accs):
            merge_pairs.append((r, a))
    return dict(S=S, off=off, cores=cores, T_pad=T_pad, runs=runs,
                n_runs=n_runs, n_acc=n_acc, merge_pairs=merge_pairs,
                t_real=t_real)


def build_core_inputs(pl, atom_features, Wa, Wo, bo):
    A = np.asarray(atom_features, dtype=np.float32)
    Wa = np.asarray(Wa, dtype=np.float32).reshape(-1)
    Wo = np.asarray(Wo, dtype=np.float32)
    bo = np.asarray(bo, dtype=np.float32).reshape(1, -1)
    D = A.shape[1]
    Dp = D + 2
    T_pad, runs, n_runs = pl["T_pad"], pl["runs"], pl["n_runs"]
    merge_pairs, n_acc = pl["merge_pairs"], pl["n_acc"]

    wa_rep = np.zeros((P, Dp), dtype=np.float32)
    wa_rep[:, :D] = Wa[None, :]
    wa_cols = np.stack([Wa[:P], Wa[P:]], axis=1).astype(np.float32)  # [128,2]
    wo_packed = np.concatenate([Wo[:P, :], Wo[P:, :]], axis=1)
    ident = np.eye(P, dtype=np.float32)
    iota = np.tile(np.arange(P, dtype=np.float32)[None, :], (P, 1))

    run_of_tile = np.zeros(T_pad, dtype=np.int64)
    for r, (t0, t1) in enumerate(runs):
        run_of_tile[t0:t1] = r

    in_maps = []
    for cc in pl["cores"]:
        a0, a1 = cc["a0"], cc["a1"]
        na = a1 - a0
        a_aug = np.zeros((T_pad * P, Dp), dtype=np.float32)
        a_aug[:na, :D] = A[a0:a1]
        a_aug[:na, D] = 1.0

        dcols = np.full((T_pad, P), -1.0, dtype=np.float32)
        seg_atom = cc["seg_atom"]
        for t in range(T_pad):
            lo, hi = t * P, min((t + 1) * P, na)
            if lo >= hi:
                continue
            w = cc["W_run"][run_of_tile[t]]
            d = seg_atom[lo:hi] - w
            assert d.min() >= 0 and d.max() < P
            dcols[t, :hi - lo] = d.astype(np.float32)
        dcols_packed = (dcols.reshape(T_pad // TB, TB, P)
                        .transpose(0, 2, 1).copy())   # [blocks, 128, TB]

        # merge matrices: window row w of run r -> acc row (W_run + w) - 128*a
        mm = np.zeros((len(merge_pairs), P, P), dtype=np.float32)
        for k, (r, a) in enumerate(merge_pairs):
            wbase = cc["W_run"][r]
            t0, t1 = runs[r]
            fs = cc["first_seg"][t0:t1]
            if not (fs >= 0).any():
                continue
            hi = int(cc["last_seg"][t0:t1][fs >= 0].max())
            for wrow in range(min(P, hi - wbase + 1)):
                g = wbase + wrow
                if a * P <= g < (a + 1) * P:
                    mm[k, wrow, g - a * P] = 1.0

        in_maps.append({
            "a_aug": a_aug, "dcols": dcols_packed, "mergem": mm,
            "wa_rep": wa_rep, "wa_cols": wa_cols,
            "wo_packed": wo_packed, "bo_row": bo,
            "ident": ident, "identr": ident, "iota": iota,
        })
    return in_maps


def build_program(pl, D=256, use_f32r=USE_F32R):
    bass, bacc, mybir, tile = _imports()
    from contextlib import ExitStack
    f32 = mybir.dt.float32
    fmm = mybir.dt.float32r if use_f32r else f32
    Dp = D + 2
    T_pad, runs, n_runs = pl["T_pad"], pl["runs"], pl["n_runs"]
    merge_pairs, n_acc = pl["merge_pairs"], pl["n_acc"]
    n_mm = len(merge_pairs)

    run_of_tile = {}
    for r, (t0, t1) in enumerate(runs):
        for t in range(t0, t1):
            run_of_tile[t] = r

    nc = bacc.Bacc("TRN2", target_bir_lowering=False, debug=False,
                   num_devices=NCORES)
    a_aug_d = nc.dram_tensor("a_aug", [T_pad * P, Dp], fmm, kind="ExternalInput")
    dcols_d = nc.dram_tensor("dcols", [T_pad // TB, P, TB], f32, kind="ExternalInput")
    mergem_d = nc.dram_tensor("mergem", [n_mm, P, P], f32, kind="ExternalInput")
    wa_rep_d = nc.dram_tensor("wa_rep", [P, Dp], fmm, kind="ExternalInput")
    wa_cols_d = nc.dram_tensor("wa_cols", [P, 2], fmm, kind="ExternalInput")
    wo_d = nc.dram_tensor("wo_packed", [P, 2 * D], f32, kind="ExternalInput")
    bo_d = nc.dram_tensor("bo_row", [1, D], f32, kind="ExternalInput")
    ident_d = nc.dram_tensor("ident", [P, P], f32, kind="ExternalInput")
    identr_d = nc.dram_tensor("identr", [P, P], fmm, kind="ExternalInput")
    iota_d = nc.dram_tensor("iota", [P, P], f32, kind="ExternalInput")
    out_d = nc.dram_tensor("out", [n_acc * P, D], f32, kind="ExternalOutput")
    scr_d = nc.dram_tensor("lscratch", [T_pad // 4, 4 * P], f32)

    with ExitStack() as top:
        tc = top.enter_context(tile.TileContext(nc))
        consts = top.enter_context(tc.tile_pool(name="consts", bufs=1))
        wa_rep = consts.tile([P, Dp], fmm)
        nc.sync.dma_start(wa_rep[:], wa_rep_d[:])
        wa_cols = consts.tile([P, 2], fmm)
        nc.sync.dma_start(wa_cols[:], wa_cols_d[:])
        wo_sb = consts.tile([P, 2 * D], f32)
        nc.sync.dma_start(wo_sb[:], wo_d[:])
        bo_sb = consts.tile([1, D], f32)
        nc.sync.dma_start(bo_sb[:], bo_d[:])
        ident = consts.tile([P, P], f32)
        nc.sync.dma_start(ident[:], ident_d[:])
        identr = consts.tile([P, P], fmm)
        nc.sync.dma_start(identr[:], identr_d[:])
        iota = consts.tile([P, P], f32)
        nc.sync.dma_start(iota[:], iota_d[:])
        msb = consts.tile([P, n_mm * P], f32)
        nc.sync.dma_start(
            msb[:].rearrange("p (k m) -> p k m", k=n_mm),
            mergem_d[:].rearrange("k p m -> p k m"))
        ones1 = consts.tile([1, P], f32)
        nc.gpsimd.memset(ones1[:], 1.0)
        flush = consts.tile([P, n_runs * Dp], f32)

        with ExitStack() as loop:
            accp = loop.enter_context(tc.tile_pool(name="accr", bufs=2, space="PSUM"))
            apool = loop.enter_context(tc.tile_pool(name="a", bufs=3))
            dpool2 = loop.enter_context(tc.tile_pool(name="dc", bufs=3))
            cpool = loop.enter_context(tc.tile_pool(name="cols", bufs=3))
            epool = loop.enter_context(tc.tile_pool(name="ecols", bufs=3))
            dpool = loop.enter_context(tc.tile_pool(name="dump", bufs=3))
            Epool = loop.enter_context(tc.tile_pool(name="Ew", bufs=18))
            tpp2 = loop.enter_context(tc.tile_pool(name="tpx", bufs=4, space="PSUM"))
            lrowp = loop.enter_context(tc.tile_pool(name="lrow", bufs=2, space="PSUM"))
            atp = loop.enter_context(tc.tile_pool(name="at", bufs=4))
            erp = loop.enter_context(tc.tile_pool(name="erow", bufs=3))

            a_view = a_aug_d[:].rearrange("(b t p) f -> b p t f", t=TB, p=P)
            state = {"accr": None}
            pending = []

            def emit_emm(pb):
                b, a_st, ews = pb
                for tl in range(TB):
                    t = b * TB + tl
                    r = run_of_tile[t]
                    t0, t1 = runs[r]
                    if t == t0:
                        state["accr"] = accp.tile([P, Dp], f32, name="accr")
                    accr = state["accr"]
                    nc.tensor.matmul(accr[:], lhsT=ews[tl][:],
                                     rhs=a_st[:, tl * Dp:(tl + 1) * Dp],
                                     start=(t == t0), stop=(t == t1 - 1))
                    if t == t1 - 1:
                        nc.scalar.copy(flush[:, r * Dp:(r + 1) * Dp], accr[:])

            for b in range(T_pad // TB):
                a_st = apool.tile([P, TB * Dp], fmm)
                nc.sync.dma_start(a_st[:].rearrange("p (t f) -> p t f", t=TB),
                                  a_view[b])
                import os as _os
                d_st = dpool2.tile([P, TB], f32)
                nc.sync.dma_start(d_st[:], dcols_d[b])
                colbuf = cpool.tile([P, TB], f32)
                pe_block = (b % 3) < int(_os.environ.get("FL_PEB", "2"))
                if not pe_block:
                    for tl in range(TB):
                        dump = dpool.tile([P, Dp], f32)
                        nc.vector.tensor_mul(
                            dump[:], a_st[:, tl * Dp:(tl + 1) * Dp], wa_rep[:])
                        nc.vector.reduce_sum(
                            colbuf[:, tl:tl + 1], dump[:],
                            axis=mybir.AxisListType.X)
                for g in range(TB // 4 if pe_block else 0):
                    tpA = tpp2.tile([P, 4 * P], f32, name="tpA", tag="tp")
                    tpB = tpp2.tile([P, 4 * P], f32, name="tpB", tag="tp")
                    for gi in range(4):
                        tl = g * 4 + gi
                        base = tl * Dp
                        nc.tensor.transpose(
                            tpA[:, gi * P:(gi + 1) * P],
                            a_st[:, base:base + P].bitcast(f32), ident[:])
                        nc.tensor.transpose(
                            tpB[:, gi * P:(gi + 1) * P],
                            a_st[:, base + P:base + 2 * P].bitcast(f32), ident[:])
                    atA = atp.tile([P, 4 * P], fmm, name="atA", tag="at")
                    nc.vector.tensor_copy(atA[:], tpA[:])
                    atB = atp.tile([P, 4 * P], fmm, name="atB", tag="at")
                    nc.scalar.copy(atB[:], tpB[:])
                    lrow = lrowp.tile([1, 4 * P], f32)
                    nc.tensor.matmul(lrow[:], lhsT=wa_cols[:, 0:1], rhs=atA[:],
                                     start=True, stop=False)
                    nc.tensor.matmul(lrow[:], lhsT=wa_cols[:, 1:2], rhs=atB[:],
                                     start=False, stop=True)
                    erow = erp.tile([1, 4 * P], f32)
                    nc.scalar.copy(erow[:], lrow[:])
                    # reshape [1, 4*128] row -> [128, 4] cols via DRAM bounce
                    gg = b * (TB // 4) + g
                    nc.sync.dma_start(scr_d[gg:gg + 1, :], erow[:])
                    nc.sync.dma_start(
                        colbuf[:, g * 4:(g + 1) * 4],
                        scr_d[gg:gg + 1, :].rearrange("o (t p) -> (o p) t", p=P))
                ecols = epool.tile([P, TB], f32)
                nc.scalar.activation(ecols[:], colbuf[:],
                                     mybir.ActivationFunctionType.Exp)
                ews = []
                for tl in range(TB):
                    Ew = Epool.tile([P, P], fmm, name="Ew")
                    nc.vector.tensor_scalar(
                        Ew[:], iota[:], d_st[:, tl:tl + 1],
                        ecols[:, tl:tl + 1],
                        op0=mybir.AluOpType.is_equal,
                        op1=mybir.AluOpType.mult)
                    ews.append(Ew)
                pending.append((b, a_st, ews))
                if len(pending) > 1:
                    emit_emm(pending.pop(0))
            for pb in pending:
                emit_emm(pb)

        # ---- merge windows into global accumulators
        with ExitStack() as tail:
            acc2p = tail.enter_context(tc.tile_pool(name="acc2", bufs=1, space="PSUM"))
            acc2 = [acc2p.tile([P, Dp], f32, name=f"acc2_{i}")
                    for i in range(n_acc)]
            first = {a: True for a in range(n_acc)}
            last_k = {}
            for k, (r, a) in enumerate(merge_pairs):
                last_k[a] = k
            import os as _os
            if _os.environ.get("FL_EMM", "1") == "1" and _os.environ.get("FL_MERGE", "1") == "1":
                for k, (r, a) in enumerate(merge_pairs):
                    nc.tensor.matmul(acc2[a][:],
                                     lhsT=msb[:, k * P:(k + 1) * P],
                                     rhs=flush[:, r * Dp:(r + 1) * Dp],
                                     start=first[a], stop=(last_k[a] == k))
                    first[a] = False
            else:
                for a in range(n_acc):
                    nc.tensor.matmul(acc2[a][:], lhsT=ones1[:],
                                     rhs=flush[0:1, 0:Dp],
                                     start=True, stop=True)

            spool = tail.enter_context(tc.tile_pool(name="small", bufs=3 * n_acc))
            tpp = tail.enter_context(tc.tile_pool(name="tp", bufs=2, space="PSUM"))
            ptsb = [consts.tile([P, n_acc * P], f32, name=f"ptsb{h}")
                    for h in range(2)]
            for a in range(n_acc):
                dsafe = spool.tile([P, 1], f32)
                nc.vector.tensor_scalar_max(dsafe[:], acc2[a][:, D:D + 1], 1e-30)
                recip = spool.tile([P, 1], f32)
                nc.vector.reciprocal(recip[:], dsafe[:])
                pooledN = spool.tile([P, D], f32)
                nc.vector.tensor_scalar_mul(pooledN[:], acc2[a][:, 0:D], recip[:])
                for h in range(2):
                    tp = tpp.tile([P, P], f32)
                    nc.tensor.transpose(tp[:], pooledN[:, h * P:(h + 1) * P],
                                        ident[:])
                    nc.scalar.copy(ptsb[h][:, a * P:(a + 1) * P], tp[:])
            opp = tail.enter_context(tc.tile_pool(name="outp", bufs=2, space="PSUM"))
            opool = tail.enter_context(tc.tile_pool(name="osb", bufs=2))
            for a in range(n_acc):
                outp = opp.tile([P, D], f32)
                for h in range(2):
                    nc.tensor.matmul(outp[:], lhsT=ptsb[h][:, a * P:(a + 1) * P],
                                     rhs=wo_sb[:, h * D:(h + 1) * D],
                                     start=(h == 0), stop=False)
                nc.tensor.matmul(outp[:], lhsT=ones1[:], rhs=bo_sb[:],
                                 start=False, stop=True)
                outsb = opool.tile([P, D], f32)
                nc.vector.tensor_copy(outsb[:], outp[:])
                nc.sync.dma_start(out_d[a * P:(a + 1) * P, :], outsb[:])
    nc.compile()
    return nc


def kernel(atom_features, index_list, Wa, ba, Wo, bo):
    atom_features = np.asarray(atom_features, dtype=np.float32)
    index_list = np.asarray(index_list)
    n_atoms, D = atom_features.shape
    pl = plan(index_list, n_atoms)
    in_maps = build_core_inputs(pl, atom_features, Wa, Wo, bo)
    nc = build_program(pl, D=D, use_f32r=USE_F32R)

    from concourse.bass_utils import run_bass_kernel_spmd
    res = run_bass_kernel_spmd(nc, in_maps, list(range(NCORES)))
    outs = []
    for c, cc in enumerate(pl["cores"]):
        outs.append(res.results[c]["out"][:cc["n_seg"]])
    return np.concatenate(outs, axis=0)


# revision 39
# speedup vs baseline: 1.1706x; 1.1700x over previous
"""Trainium2 Bass kernel for segment-wise self-attention pooling.

Computation (per segment s of a ragged atom array):
  logits = A @ Wa ; e = exp(logits) (softmax shift-invariant, logits are O(+-5))
  att    = e / segsum(e) ; pooled[s] = sum_{i in s} att_i * A_i
  out[s] = pooled[s] @ Wo + bo

Strategy (8 NeuronCores, SPMD — one program, per-core data), v2:
  - Shard atoms by contiguous segment ranges, balanced by atom count.
  - Tiles are grouped into RUNS (program-static tile ranges). Within a run,
    every tile's segments fit a per-core 128-segment window [W_run, W_run+128)
    (host-validated). Per-atom window column d_p = seg(p) - W_run is shipped
    as data ([128,1] f32 per tile).
  - Per tile: logits on DVE (mul+reduce vs replicated Wa) or on PE
    (f32r transposes + flipped matmul), batched ACT exp, then ONE op builds
    the full attention matrix E_wide[p,j] = (iota[j]==d_p)*e_p from a
    constant iota tile (no mask DMA). PE matmul E_wide.T @ [A|1|0] (float32r,
    1 cyc/row, base 0) accumulates straight into the run's PSUM window.
  - Run flush: one ACT copy PSUM->SBUF per run (~3/core). Final merge maps
    window rows to global segment rows with small host-built 0/1 matrices
    (data) via a few fp32 matmuls; then reciprocal, scale, PE transpose,
    project with Wo, +bo via K=1 ones matmul, DMA out. Host concatenates.
"""

import sys
import numpy as np

sys.path.insert(0, "/opt/trn_rl_repo")

P = 128
TB = 8            # tiles per DMA block
MAXS = 8          # max segments one tile may touch (validated)
WMAX = 120        # window capacity used during run construction
NCORES = 8
USE_F32R = True
PE_LOGITS_MOD8 = 0   # tiles with (t%8) < this use the PE logits path


def _imports():
    import concourse.bass as bass
    import concourse.bacc as bacc
    import concourse.mybir as mybir
    import concourse.tile as tile
    return bass, bacc, mybir, tile


def plan(index_list, n_atoms, n_cores=NCORES):
    off = np.asarray(index_list, dtype=np.int64)
    S = len(off) - 1
    targets = (np.arange(1, n_cores) * n_atoms) // n_cores
    cuts = np.searchsorted(off, targets)
    for i, t in enumerate(targets):
        c = cuts[i]
        if c > 0 and abs(off[c - 1] - t) < abs(off[c] - t):
            cuts[i] = c - 1
    seg_cuts = np.concatenate([[0], cuts, [S]])
    cores = []
    for c in range(n_cores):
        s0, s1 = int(seg_cuts[c]), int(seg_cuts[c + 1])
        a0, a1 = int(off[s0]), int(off[s1])
        cores.append(dict(s0=s0, s1=s1, a0=a0, a1=a1,
                          n_seg=s1 - s0, n_atom=a1 - a0))
    t_real = [int(np.ceil(cc["n_atom"] / P)) for cc in cores]
    T_pad = int(np.ceil(max(t_real) / TB) * TB)

    for cc in cores:
        a0, a1, s0 = cc["a0"], cc["a1"], cc["s0"]
        seg_atom = np.searchsorted(off, np.arange(a0, a1), side="right") - 1 - s0
        cc["seg_atom"] = seg_atom
        first_seg = np.full(T_pad, -1, dtype=np.int64)
        last_seg = np.full(T_pad, -1, dtype=np.int64)
        for t in range(T_pad):
            lo, hi = t * P, min((t + 1) * P, a1 - a0)
            if lo >= hi:
                continue
            first_seg[t] = seg_atom[lo]
            last_seg[t] = seg_atom[hi - 1]
        cc["first_seg"] = first_seg
        cc["last_seg"] = last_seg
        ok = first_seg >= 0
        assert (last_seg[ok] - first_seg[ok]).max() + 1 <= MAXS

    # ---- runs: maximal tile ranges where every core's segs fit one window
    runs = []
    start = 0
    while start < T_pad:
        end = start + 1
        while end < T_pad:
            fits = True
            for cc in cores:
                fs = cc["first_seg"][start:end + 1]
                ls = cc["last_seg"][start:end + 1]
                m = fs >= 0
                if m.any() and ls[m].max() - fs[m].min() + 1 > WMAX:
                    fits = False
                    break
            if not fits:
                break
            end += 1
        runs.append((start, end))
        start = end
    n_runs = len(runs)

    # per-core per-run window base; merge structure
    n_acc = max(int(np.ceil(cc["n_seg"] / P)) for cc in cores)
    merge_pairs = []   # program-static (run, acc_tile)
    for r, (t0, t1) in enumerate(runs):
        accs = set()
        for cc in cores:
            fs = cc["first_seg"][t0:t1]
            m = fs >= 0
            if not m.any():
                cc.setdefault("W_run", []).append(0)
                continue
            w = int(fs[m].min())
            hi = int(cc["last_seg"][t0:t1][m].max())
            assert hi - w < P
            cc.setdefault("W_run", []).append(w)
            accs.add(min(w // P, n_acc - 1))
            if (w + P - 1) // P != w // P:
                a2 = min((w + P - 1) // P, n_acc - 1)
                accs.add(a2)
            if hi // P != w // P:
                accs.add(min(hi // P, n_acc - 1))
        for a in sorted(accs):
            merge_pairs.append((r, a))
    return dict(S=S, off=off, cores=cores, T_pad=T_pad, runs=runs,
                n_runs=n_runs, n_acc=n_acc, merge_pairs=merge_pairs,
                t_real=t_real)


def build_core_inputs(pl, atom_features, Wa, Wo, bo):
    A = np.asarray(atom_features, dtype=np.float32)
    Wa = np.asarray(Wa, dtype=np.float32).reshape(-1)
    Wo = np.asarray(Wo, dtype=np.float32)
    bo = np.asarray(bo, dtype=np.float32).reshape(1, -1)
    D = A.shape[1]
    Dp = D + 2
    T_pad, runs, n_runs = pl["T_pad"], pl["runs"], pl["n_runs"]
    merge_pairs, n_acc = pl["merge_pairs"], pl["n_acc"]

    wa_rep = np.zeros((P, Dp), dtype=np.float32)
    wa_rep[:, :D] = Wa[None, :]
    wa_cols = np.stack([Wa[:P], Wa[P:]], axis=1).astype(np.float32)  # [128,2]
    wo_packed = np.concatenate([Wo[:P, :], Wo[P:, :]], axis=1)
    ident = np.eye(P, dtype=np.float32)
    iota = np.tile(np.arange(P, dtype=np.float32)[None, :], (P, 1))

    run_of_tile = np.zeros(T_pad, dtype=np.int64)
    for r, (t0, t1) in enumerate(runs):
        run_of_tile[t0:t1] = r

    in_maps = []
    for cc in pl["cores"]:
        a0, a1 = cc["a0"], cc["a1"]
        na = a1 - a0
        a_aug = np.zeros((T_pad * P, Dp), dtype=np.float32)
        a_aug[:na, :D] = A[a0:a1]
        a_aug[:na, D] = 1.0

        dcols = np.full((T_pad, P), -1.0, dtype=np.float32)
        seg_atom = cc["seg_atom"]
        for t in range(T_pad):
            lo, hi = t * P, min((t + 1) * P, na)
            if lo >= hi:
                continue
            w = cc["W_run"][run_of_tile[t]]
            d = seg_atom[lo:hi] - w
            assert d.min() >= 0 and d.max() < P
            dcols[t, :hi - lo] = d.astype(np.float32)
        dcols_packed = (dcols.reshape(T_pad // TB, TB, P)
                        .transpose(0, 2, 1).copy())   # [blocks, 128, TB]

        # merge matrices: window row w of run r -> acc row (W_run + w) - 128*a
        mm = np.zeros((len(merge_pairs), P, P), dtype=np.float32)
        for k, (r, a) in enumerate(merge_pairs):
            wbase = cc["W_run"][r]
            t0, t1 = runs[r]
            fs = cc["first_seg"][t0:t1]
            if not (fs >= 0).any():
                continue
            hi = int(cc["last_seg"][t0:t1][fs >= 0].max())
            for wrow in range(min(P, hi - wbase + 1)):
                g = wbase + wrow
                if a * P <= g < (a + 1) * P:
                    mm[k, wrow, g - a * P] = 1.0

        in_maps.append({
            "a_aug": a_aug, "dcols": dcols_packed, "mergem": mm,
            "wa_rep": wa_rep, "wa_cols": wa_cols,
            "wo_packed": wo_packed, "bo_row": bo,
            "ident": ident, "identr": ident, "iota": iota,
        })
    return in_maps


def build_program(pl, D=256, use_f32r=USE_F32R):
    bass, bacc, mybir, tile = _imports()
    from contextlib import ExitStack
    f32 = mybir.dt.float32
    fmm = mybir.dt.float32r if use_f32r else f32
    Dp = D + 2
    T_pad, runs, n_runs = pl["T_pad"], pl["runs"], pl["n_runs"]
    merge_pairs, n_acc = pl["merge_pairs"], pl["n_acc"]
    n_mm = len(merge_pairs)

    run_of_tile = {}
    for r, (t0, t1) in enumerate(runs):
        for t in range(t0, t1):
            run_of_tile[t] = r

    nc = bacc.Bacc("TRN2", target_bir_lowering=False, debug=False,
                   num_devices=NCORES)
    a_aug_d = nc.dram_tensor("a_aug", [T_pad * P, Dp], fmm, kind="ExternalInput")
    dcols_d = nc.dram_tensor("dcols", [T_pad // TB, P, TB], f32, kind="ExternalInput")
    mergem_d = nc.dram_tensor("mergem", [n_mm, P, P], f32, kind="ExternalInput")
    wa_rep_d = nc.dram_tensor("wa_rep", [P, Dp], fmm, kind="ExternalInput")
    wa_cols_d = nc.dram_tensor("wa_cols", [P, 2], fmm, kind="ExternalInput")
    wo_d = nc.dram_tensor("wo_packed", [P, 2 * D], f32, kind="ExternalInput")
    bo_d = nc.dram_tensor("bo_row", [1, D], f32, kind="ExternalInput")
    ident_d = nc.dram_tensor("ident", [P, P], f32, kind="ExternalInput")
    identr_d = nc.dram_tensor("identr", [P, P], fmm, kind="ExternalInput")
    iota_d = nc.dram_tensor("iota", [P, P], f32, kind="ExternalInput")
    out_d = nc.dram_tensor("out", [n_acc * P, D], f32, kind="ExternalOutput")
    scr_d = nc.dram_tensor("lscratch", [T_pad // 4, 4 * P], f32)

    with ExitStack() as top:
        tc = top.enter_context(tile.TileContext(nc))
        consts = top.enter_context(tc.tile_pool(name="consts", bufs=1))
        wa_rep = consts.tile([P, Dp], fmm)
        nc.sync.dma_start(wa_rep[:], wa_rep_d[:])
        wa_cols = consts.tile([P, 2], fmm)
        nc.sync.dma_start(wa_cols[:], wa_cols_d[:])
        wo_sb = consts.tile([P, 2 * D], f32)
        nc.sync.dma_start(wo_sb[:], wo_d[:])
        bo_sb = consts.tile([1, D], f32)
        nc.sync.dma_start(bo_sb[:], bo_d[:])
        ident = consts.tile([P, P], f32)
        nc.sync.dma_start(ident[:], ident_d[:])
        identr = consts.tile([P, P], fmm)
        nc.sync.dma_start(identr[:], identr_d[:])
        iota = consts.tile([P, P], f32)
        nc.sync.dma_start(iota[:], iota_d[:])
        msb = consts.tile([P, n_mm * P], f32)
        nc.sync.dma_start(
            msb[:].rearrange("p (k m) -> p k m", k=n_mm),
            mergem_d[:].rearrange("k p m -> p k m"))
        ones1 = consts.tile([1, P], f32)
        nc.gpsimd.memset(ones1[:], 1.0)
        flush = consts.tile([P, n_runs * Dp], f32)

        with ExitStack() as loop:
            accp = loop.enter_context(tc.tile_pool(name="accr", bufs=2, space="PSUM"))
            apool = loop.enter_context(tc.tile_pool(name="a", bufs=3))
            dpool2 = loop.enter_context(tc.tile_pool(name="dc", bufs=3))
            cpool = loop.enter_context(tc.tile_pool(name="cols", bufs=3))
            epool = loop.enter_context(tc.tile_pool(name="ecols", bufs=3))
            dpool = loop.enter_context(tc.tile_pool(name="dump", bufs=3))
            Epool = loop.enter_context(tc.tile_pool(name="Ew", bufs=18))
            tpp2 = loop.enter_context(tc.tile_pool(name="tpx", bufs=4, space="PSUM"))
            lrowp = loop.enter_context(tc.tile_pool(name="lrow", bufs=2, space="PSUM"))
            atp = loop.enter_context(tc.tile_pool(name="at", bufs=4))
            erp = loop.enter_context(tc.tile_pool(name="erow", bufs=3))

            a_view = a_aug_d[:].rearrange("(b t p) f -> b p t f", t=TB, p=P)
            state = {"accr": None}
            pending = []

            def emit_emm(pb):
                b, a_st, ews = pb
                for tl in range(TB):
                    t = b * TB + tl
                    r = run_of_tile[t]
                    t0, t1 = runs[r]
                    if t == t0:
                        state["accr"] = accp.tile([P, Dp], f32, name="accr")
                    accr = state["accr"]
                    nc.tensor.matmul(accr[:], lhsT=ews[tl][:],
                                     rhs=a_st[:, tl * Dp:(tl + 1) * Dp],
                                     start=(t == t0), stop=(t == t1 - 1))
                    if t == t1 - 1:
                        nc.scalar.copy(flush[:, r * Dp:(r + 1) * Dp], accr[:])

            for b in range(T_pad // TB):
                a_st = apool.tile([P, TB * Dp], fmm)
                nc.sync.dma_start(a_st[:].rearrange("p (t f) -> p t f", t=TB),
                                  a_view[b])
                import os as _os
                d_st = dpool2.tile([P, TB], f32)
                nc.sync.dma_start(d_st[:], dcols_d[b])
                colbuf = cpool.tile([P, TB], f32)
                pe_block = (b % 3) < int(_os.environ.get("FL_PEB", "2"))
                if not pe_block:
                    for tl in range(TB):
                        dump = dpool.tile([P, Dp], f32)
                        nc.vector.tensor_mul(
                            dump[:], a_st[:, tl * Dp:(tl + 1) * Dp], wa_rep[:])
                        nc.vector.reduce_sum(
                            colbuf[:, tl:tl + 1], dump[:],
                            axis=mybir.AxisListType.X)
                for g in range(TB // 4 if pe_block else 0):
                    tpA = tpp2.tile([P, 4 * P], f32, name="tpA", tag="tp")
                    tpB = tpp2.tile([P, 4 * P], f32, name="tpB", tag="tp")
                    for gi in range(4):
                        tl = g * 4 + gi
                        base = tl * Dp
                        nc.tensor.transpose(
                            tpA[:, gi * P:(gi + 1) * P],
                            a_st[:, base:base + P].bitcast(f32), ident[:])
                        nc.tensor.transpose(
                            tpB[:, gi * P:(gi + 1) * P],
                            a_st[:, base + P:base + 2 * P].bitcast(f32), ident[:])
                    atA = atp.tile([P, 4 * P], fmm, name="atA", tag="at")
                    nc.vector.tensor_copy(atA[:], tpA[:])
                    atB = atp.tile([P, 4 * P], fmm, name="atB", tag="at")
                    nc.scalar.copy(atB[:], tpB[:])
                    lrow = lrowp.tile([1, 4 * P], f32)
                    nc.tensor.matmul(lrow[:], lhsT=wa_cols[:, 0:1], rhs=atA[:],
                                     start=True, stop=False)
                    nc.tensor.matmul(lrow[:], lhsT=wa_cols[:, 1:2], rhs=atB[:],
                                     start=False, stop=True)
                    erow = erp.tile([1, 4 * P], f32)
                    nc.scalar.copy(erow[:], lrow[:])
                    # reshape [1, 4*128] row -> [128, 4] cols via DRAM bounce
                    gg = b * (TB // 4) + g
                    nc.sync.dma_start(scr_d[gg:gg + 1, :], erow[:])
                    nc.sync.dma_start(
                        colbuf[:, g * 4:(g + 1) * 4],
                        scr_d[gg:gg + 1, :].rearrange("o (t p) -> (o p) t", p=P))
                ecols = epool.tile([P, TB], f32)
                nc.scalar.activation(ecols[:], colbuf[:],
                                     mybir.ActivationFunctionType.Exp)
                ews = []
                for tl in range(TB):
                    Ew = Epool.tile([P, P], fmm, name="Ew")
                    nc.vector.tensor_scalar(
                        Ew[:], iota[:], d_st[:, tl:tl + 1],
                        ecols[:, tl:tl + 1],
                        op0=mybir.AluOpType.is_equal,
                        op1=mybir.AluOpType.mult)
                    ews.append(Ew)
                pending.append((b, a_st, ews))
                if len(pending) > int(_os.environ.get("FL_PIPE", "1")):
                    emit_emm(pending.pop(0))
            for pb in pending:
                emit_emm(pb)

        # ---- merge windows into global accumulators
        with ExitStack() as tail:
            acc2p = tail.enter_context(tc.tile_pool(name="acc2", bufs=1, space="PSUM"))
            acc2 = [acc2p.tile([P, Dp], f32, name=f"acc2_{i}")
                    for i in range(n_acc)]
            first = {a: True for a in range(n_acc)}
            last_k = {}
            for k, (r, a) in enumerate(merge_pairs):
                last_k[a] = k
            import os as _os
            if _os.environ.get("FL_EMM", "1") == "1" and _os.environ.get("FL_MERGE", "1") == "1":
                for k, (r, a) in enumerate(merge_pairs):
                    nc.tensor.matmul(acc2[a][:],
                                     lhsT=msb[:, k * P:(k + 1) * P],
                                     rhs=flush[:, r * Dp:(r + 1) * Dp],
                                     start=first[a], stop=(last_k[a] == k))
                    first[a] = False
            else:
                for a in range(n_acc):
                    nc.tensor.matmul(acc2[a][:], lhsT=ones1[:],
                                     rhs=flush[0:1, 0:Dp],
                                     start=True, stop=True)

            spool = tail.enter_context(tc.tile_pool(name="small", bufs=3 * n_acc))
            tpp = tail.enter_context(tc.tile_pool(name="tp", bufs=2, space="PSUM"))
            ptsb = [consts.tile([P, n_acc * P], f32, name=f"ptsb{h}")
                    for h in range(2)]
            for a in range(n_acc):
                dsafe = spool.tile([P, 1], f32)
                nc.vector.tensor_scalar_max(dsafe[:], acc2[a][:, D:D + 1], 1e-30)
                recip = spool.tile([P, 1], f32)
                nc.vector.reciprocal(recip[:], dsafe[:])
                pooledN = spool.tile([P, D], f32)
                nc.vector.tensor_scalar_mul(pooledN[:], acc2[a][:, 0:D], recip[:])
                for h in range(2):
                    tp = tpp.tile([P, P], f32)
                    nc.tensor.transpose(tp[:], pooledN[:, h * P:(h + 1) * P],
                                        ident[:])
                    nc.scalar.copy(ptsb[h][:, a * P:(a + 1) * P], tp[:])
            opp = tail.enter_context(tc.tile_pool(name="outp", bufs=2, space="PSUM"))
            opool = tail.enter_context(tc.tile_pool(name="osb", bufs=2))
            for a in range(n_acc):
                outp = opp.tile([P, D], f32)
                for h in range(2):
                    nc.tensor.matmul(outp[:], lhsT=ptsb[h][:, a * P:(a + 1) * P],
                                     rhs=wo_sb[:, h * D:(h + 1) * D],
                                     start=(h == 0), stop=False)
                nc.tensor.matmul(outp[:], lhsT=ones1[:], rhs=bo_sb[:],
                                 start=False, stop=True)
                outsb = opool.tile([P, D], f32)
                nc.vector.tensor_copy(outsb[:], outp[:])
                nc.sync.dma_start(out_d[a * P:(a + 1) * P, :], outsb[:])
    nc.compile()
    return nc


def kernel(atom_features, index_list, Wa, ba, Wo, bo):
    atom_features = np.asarray(atom_features, dtype=np.float32)
    index_list = np.asarray(index_list)
    n_atoms, D = atom_features.shape
    pl = plan(index_list, n_atoms)
    in_maps = build_core_inputs(pl, atom_features, Wa, Wo, bo)
    nc = build_program(pl, D=D, use_f32r=USE_F32R)

    from concourse.bass_utils import run_bass_kernel_spmd
    res = run_bass_kernel_spmd(nc, in_maps, list(range(NCORES)))
    outs = []
    for c, cc in enumerate(pl["cores"]):
        outs.append(res.results[c]["out"][:cc["n_seg"]])
    return np.concatenate(outs, axis=0)


# revision 40
# speedup vs baseline: 1.1986x; 1.0239x over previous
"""Trainium2 Bass kernel for segment-wise self-attention pooling.

Computation (per segment s of a ragged atom array):
  logits = A @ Wa ; e = exp(logits) (softmax shift-invariant, logits are O(+-5))
  att    = e / segsum(e) ; pooled[s] = sum_{i in s} att_i * A_i
  out[s] = pooled[s] @ Wo + bo

Strategy (8 NeuronCores, SPMD — one program, per-core data), v2:
  - Shard atoms by contiguous segment ranges, balanced by atom count.
  - Tiles are grouped into RUNS (program-static tile ranges). Within a run,
    every tile's segments fit a per-core 128-segment window [W_run, W_run+128)
    (host-validated). Per-atom window column d_p = seg(p) - W_run is shipped
    as data ([128,1] f32 per tile).
  - Per tile: logits on DVE (mul+reduce vs replicated Wa) or on PE
    (f32r transposes + flipped matmul), batched ACT exp, then ONE op builds
    the full attention matrix E_wide[p,j] = (iota[j]==d_p)*e_p from a
    constant iota tile (no mask DMA). PE matmul E_wide.T @ [A|1|0] (float32r,
    1 cyc/row, base 0) accumulates straight into the run's PSUM window.
  - Run flush: one ACT copy PSUM->SBUF per run (~3/core). Final merge maps
    window rows to global segment rows with small host-built 0/1 matrices
    (data) via a few fp32 matmuls; then reciprocal, scale, PE transpose,
    project with Wo, +bo via K=1 ones matmul, DMA out. Host concatenates.
"""

import sys
import numpy as np

sys.path.insert(0, "/opt/trn_rl_repo")

P = 128
TB = 8            # tiles per DMA block
MAXS = 8          # max segments one tile may touch (validated)
WMAX = 56         # window capacity used during run construction
WCOLS = 64        # E matrix / accumulator window width (M of the E-matmul)
NCORES = 8
USE_F32R = True
PE_LOGITS_MOD8 = 0   # tiles with (t%8) < this use the PE logits path


def _imports():
    import concourse.bass as bass
    import concourse.bacc as bacc
    import concourse.mybir as mybir
    import concourse.tile as tile
    return bass, bacc, mybir, tile


def plan(index_list, n_atoms, n_cores=NCORES):
    off = np.asarray(index_list, dtype=np.int64)
    S = len(off) - 1
    targets = (np.arange(1, n_cores) * n_atoms) // n_cores
    cuts = np.searchsorted(off, targets)
    for i, t in enumerate(targets):
        c = cuts[i]
        if c > 0 and abs(off[c - 1] - t) < abs(off[c] - t):
            cuts[i] = c - 1
    seg_cuts = np.concatenate([[0], cuts, [S]])
    cores = []
    for c in range(n_cores):
        s0, s1 = int(seg_cuts[c]), int(seg_cuts[c + 1])
        a0, a1 = int(off[s0]), int(off[s1])
        cores.append(dict(s0=s0, s1=s1, a0=a0, a1=a1,
                          n_seg=s1 - s0, n_atom=a1 - a0))
    t_real = [int(np.ceil(cc["n_atom"] / P)) for cc in cores]
    T_pad = int(np.ceil(max(t_real) / TB) * TB)

    for cc in cores:
        a0, a1, s0 = cc["a0"], cc["a1"], cc["s0"]
        seg_atom = np.searchsorted(off, np.arange(a0, a1), side="right") - 1 - s0
        cc["seg_atom"] = seg_atom
        first_seg = np.full(T_pad, -1, dtype=np.int64)
        last_seg = np.full(T_pad, -1, dtype=np.int64)
        for t in range(T_pad):
            lo, hi = t * P, min((t + 1) * P, a1 - a0)
            if lo >= hi:
                continue
            first_seg[t] = seg_atom[lo]
            last_seg[t] = seg_atom[hi - 1]
        cc["first_seg"] = first_seg
        cc["last_seg"] = last_seg
        ok = first_seg >= 0
        assert (last_seg[ok] - first_seg[ok]).max() + 1 <= MAXS

    # ---- runs: maximal tile ranges where every core's segs fit one window
    runs = []
    start = 0
    while start < T_pad:
        end = start + 1
        while end < T_pad:
            fits = True
            for cc in cores:
                fs = cc["first_seg"][start:end + 1]
                ls = cc["last_seg"][start:end + 1]
                m = fs >= 0
                if m.any() and ls[m].max() - fs[m].min() + 1 > WMAX:
                    fits = False
                    break
            if not fits:
                break
            end += 1
        runs.append((start, end))
        start = end
    n_runs = len(runs)

    # per-core per-run window base; merge structure
    n_acc = max(int(np.ceil(cc["n_seg"] / P)) for cc in cores)
    merge_pairs = []   # program-static (run, acc_tile)
    for r, (t0, t1) in enumerate(runs):
        accs = set()
        for cc in cores:
            fs = cc["first_seg"][t0:t1]
            m = fs >= 0
            if not m.any():
                cc.setdefault("W_run", []).append(0)
                continue
            w = int(fs[m].min())
            hi = int(cc["last_seg"][t0:t1][m].max())
            assert hi - w < WCOLS
            cc.setdefault("W_run", []).append(w)
            accs.add(min(w // P, n_acc - 1))
            if (w + P - 1) // P != w // P:
                a2 = min((w + P - 1) // P, n_acc - 1)
                accs.add(a2)
            if hi // P != w // P:
                accs.add(min(hi // P, n_acc - 1))
        for a in sorted(accs):
            merge_pairs.append((r, a))
    return dict(S=S, off=off, cores=cores, T_pad=T_pad, runs=runs,
                n_runs=n_runs, n_acc=n_acc, merge_pairs=merge_pairs,
                t_real=t_real)


def build_core_inputs(pl, atom_features, Wa, Wo, bo):
    A = np.asarray(atom_features, dtype=np.float32)
    Wa = np.asarray(Wa, dtype=np.float32).reshape(-1)
    Wo = np.asarray(Wo, dtype=np.float32)
    bo = np.asarray(bo, dtype=np.float32).reshape(1, -1)
    D = A.shape[1]
    Dp = D + 2
    T_pad, runs, n_runs = pl["T_pad"], pl["runs"], pl["n_runs"]
    merge_pairs, n_acc = pl["merge_pairs"], pl["n_acc"]

    wa_rep = np.zeros((P, Dp), dtype=np.float32)
    wa_rep[:, :D] = Wa[None, :]
    wa_cols = np.stack([Wa[:P], Wa[P:]], axis=1).astype(np.float32)  # [128,2]
    wo_packed = np.concatenate([Wo[:P, :], Wo[P:, :]], axis=1)
    ident = np.eye(P, dtype=np.float32)
    iota = np.tile(np.arange(P, dtype=np.float32)[None, :], (P, 1))

    run_of_tile = np.zeros(T_pad, dtype=np.int64)
    for r, (t0, t1) in enumerate(runs):
        run_of_tile[t0:t1] = r

    in_maps = []
    for cc in pl["cores"]:
        a0, a1 = cc["a0"], cc["a1"]
        na = a1 - a0
        a_aug = np.zeros((T_pad * P, Dp), dtype=np.float32)
        a_aug[:na, :D] = A[a0:a1]
        a_aug[:na, D] = 1.0

        dcols = np.full((T_pad, P), -1.0, dtype=np.float32)
        seg_atom = cc["seg_atom"]
        for t in range(T_pad):
            lo, hi = t * P, min((t + 1) * P, na)
            if lo >= hi:
                continue
            w = cc["W_run"][run_of_tile[t]]
            d = seg_atom[lo:hi] - w
            assert d.min() >= 0 and d.max() < WCOLS
            dcols[t, :hi - lo] = d.astype(np.float32)
        dcols_packed = (dcols.reshape(T_pad // TB, TB, P)
                        .transpose(0, 2, 1).copy())   # [blocks, 128, TB]

        # merge matrices: window row w of run r -> acc row (W_run + w) - 128*a
        mm = np.zeros((len(merge_pairs), WCOLS, P), dtype=np.float32)
        for k, (r, a) in enumerate(merge_pairs):
            wbase = cc["W_run"][r]
            t0, t1 = runs[r]
            fs = cc["first_seg"][t0:t1]
            if not (fs >= 0).any():
                continue
            hi = int(cc["last_seg"][t0:t1][fs >= 0].max())
            for wrow in range(min(WCOLS, hi - wbase + 1)):
                g = wbase + wrow
                if a * P <= g < (a + 1) * P:
                    mm[k, wrow, g - a * P] = 1.0

        in_maps.append({
            "a_aug": a_aug, "dcols": dcols_packed, "mergem": mm,
            "wa_rep": wa_rep, "wa_cols": wa_cols,
            "wo_packed": wo_packed, "bo_row": bo,
            "ident": ident, "identr": ident, "iota": iota,
        })
    return in_maps


def build_program(pl, D=256, use_f32r=USE_F32R):
    bass, bacc, mybir, tile = _imports()
    from contextlib import ExitStack
    f32 = mybir.dt.float32
    fmm = mybir.dt.float32r if use_f32r else f32
    Dp = D + 2
    T_pad, runs, n_runs = pl["T_pad"], pl["runs"], pl["n_runs"]
    merge_pairs, n_acc = pl["merge_pairs"], pl["n_acc"]
    n_mm = len(merge_pairs)

    run_of_tile = {}
    for r, (t0, t1) in enumerate(runs):
        for t in range(t0, t1):
            run_of_tile[t] = r

    nc = bacc.Bacc("TRN2", target_bir_lowering=False, debug=False,
                   num_devices=NCORES)
    a_aug_d = nc.dram_tensor("a_aug", [T_pad * P, Dp], fmm, kind="ExternalInput")
    dcols_d = nc.dram_tensor("dcols", [T_pad // TB, P, TB], f32, kind="ExternalInput")
    mergem_d = nc.dram_tensor("mergem", [n_mm, WCOLS, P], f32, kind="ExternalInput")
    wa_rep_d = nc.dram_tensor("wa_rep", [P, Dp], fmm, kind="ExternalInput")
    wa_cols_d = nc.dram_tensor("wa_cols", [P, 2], fmm, kind="ExternalInput")
    wo_d = nc.dram_tensor("wo_packed", [P, 2 * D], f32, kind="ExternalInput")
    bo_d = nc.dram_tensor("bo_row", [1, D], f32, kind="ExternalInput")
    ident_d = nc.dram_tensor("ident", [P, P], f32, kind="ExternalInput")
    identr_d = nc.dram_tensor("identr", [P, P], fmm, kind="ExternalInput")
    iota_d = nc.dram_tensor("iota", [P, P], f32, kind="ExternalInput")
    out_d = nc.dram_tensor("out", [n_acc * P, D], f32, kind="ExternalOutput")
    scr_d = nc.dram_tensor("lscratch", [T_pad // 4, 4 * P], f32)

    with ExitStack() as top:
        tc = top.enter_context(tile.TileContext(nc))
        consts = top.enter_context(tc.tile_pool(name="consts", bufs=1))
        wa_rep = consts.tile([P, Dp], fmm)
        nc.sync.dma_start(wa_rep[:], wa_rep_d[:])
        wa_cols = consts.tile([P, 2], fmm)
        nc.sync.dma_start(wa_cols[:], wa_cols_d[:])
        wo_sb = consts.tile([P, 2 * D], f32)
        nc.sync.dma_start(wo_sb[:], wo_d[:])
        bo_sb = consts.tile([1, D], f32)
        nc.sync.dma_start(bo_sb[:], bo_d[:])
        ident = consts.tile([P, P], f32)
        nc.sync.dma_start(ident[:], ident_d[:])
        identr = consts.tile([P, P], fmm)
        nc.sync.dma_start(identr[:], identr_d[:])
        iota = consts.tile([P, P], f32)
        nc.sync.dma_start(iota[:], iota_d[:])
        msb = consts.tile([WCOLS, n_mm * P], f32)
        nc.sync.dma_start(
            msb[:].rearrange("p (k m) -> p k m", k=n_mm),
            mergem_d[:].rearrange("k p m -> p k m"))
        ones1 = consts.tile([1, P], f32)
        nc.gpsimd.memset(ones1[:], 1.0)
        flush = consts.tile([WCOLS, n_runs * Dp], f32)

        with ExitStack() as loop:
            accp = loop.enter_context(tc.tile_pool(name="accr", bufs=2, space="PSUM"))
            apool = loop.enter_context(tc.tile_pool(name="a", bufs=3))
            dpool2 = loop.enter_context(tc.tile_pool(name="dc", bufs=3))
            cpool = loop.enter_context(tc.tile_pool(name="cols", bufs=3))
            epool = loop.enter_context(tc.tile_pool(name="ecols", bufs=3))
            dpool = loop.enter_context(tc.tile_pool(name="dump", bufs=3))
            Epool = loop.enter_context(tc.tile_pool(name="Ew", bufs=18))
            tpp2 = loop.enter_context(tc.tile_pool(name="tpx", bufs=4, space="PSUM"))
            lrowp = loop.enter_context(tc.tile_pool(name="lrow", bufs=2, space="PSUM"))
            atp = loop.enter_context(tc.tile_pool(name="at", bufs=4))
            erp = loop.enter_context(tc.tile_pool(name="erow", bufs=3))

            a_view = a_aug_d[:].rearrange("(b t p) f -> b p t f", t=TB, p=P)
            state = {"accr": None}
            pending = []

            def emit_emm(pb):
                b, a_st, ews = pb
                for tl in range(TB):
                    t = b * TB + tl
                    r = run_of_tile[t]
                    t0, t1 = runs[r]
                    if t == t0:
                        state["accr"] = accp.tile([WCOLS, Dp], f32, name="accr")
                    accr = state["accr"]
                    nc.tensor.matmul(accr[:], lhsT=ews[tl][:],
                                     rhs=a_st[:, tl * Dp:(tl + 1) * Dp],
                                     start=(t == t0), stop=(t == t1 - 1))
                    if t == t1 - 1:
                        nc.scalar.copy(flush[:, r * Dp:(r + 1) * Dp], accr[:])

            for b in range(T_pad // TB):
                a_st = apool.tile([P, TB * Dp], fmm)
                nc.sync.dma_start(a_st[:].rearrange("p (t f) -> p t f", t=TB),
                                  a_view[b])
                import os as _os
                d_st = dpool2.tile([P, TB], f32)
                nc.sync.dma_start(d_st[:], dcols_d[b])
                colbuf = cpool.tile([P, TB], f32)
                pe_block = (b % 3) < int(_os.environ.get("FL_PEB", "2"))
                if not pe_block:
                    for tl in range(TB):
                        dump = dpool.tile([P, Dp], f32)
                        nc.vector.tensor_mul(
                            dump[:], a_st[:, tl * Dp:(tl + 1) * Dp], wa_rep[:])
                        nc.vector.reduce_sum(
                            colbuf[:, tl:tl + 1], dump[:],
                            axis=mybir.AxisListType.X)
                for g in range(TB // 4 if pe_block else 0):
                    tpA = tpp2.tile([P, 4 * P], f32, name="tpA", tag="tp")
                    tpB = tpp2.tile([P, 4 * P], f32, name="tpB", tag="tp")
                    for gi in range(4):
                        tl = g * 4 + gi
                        base = tl * Dp
                        nc.tensor.transpose(
                            tpA[:, gi * P:(gi + 1) * P],
                            a_st[:, base:base + P].bitcast(f32), ident[:])
                        nc.tensor.transpose(
                            tpB[:, gi * P:(gi + 1) * P],
                            a_st[:, base + P:base + 2 * P].bitcast(f32), ident[:])
                    atA = atp.tile([P, 4 * P], fmm, name="atA", tag="at")
                    nc.vector.tensor_copy(atA[:], tpA[:])
                    atB = atp.tile([P, 4 * P], fmm, name="atB", tag="at")
                    nc.scalar.copy(atB[:], tpB[:])
                    lrow = lrowp.tile([1, 4 * P], f32)
                    nc.tensor.matmul(lrow[:], lhsT=wa_cols[:, 0:1], rhs=atA[:],
                                     start=True, stop=False)
                    nc.tensor.matmul(lrow[:], lhsT=wa_cols[:, 1:2], rhs=atB[:],
                                     start=False, stop=True)
                    erow = erp.tile([1, 4 * P], f32)
                    nc.scalar.copy(erow[:], lrow[:])
                    # reshape [1, 4*128] row -> [128, 4] cols via DRAM bounce
                    gg = b * (TB // 4) + g
                    nc.sync.dma_start(scr_d[gg:gg + 1, :], erow[:])
                    nc.sync.dma_start(
                        colbuf[:, g * 4:(g + 1) * 4],
                        scr_d[gg:gg + 1, :].rearrange("o (t p) -> (o p) t", p=P))
                ecols = epool.tile([P, TB], f32)
                nc.scalar.activation(ecols[:], colbuf[:],
                                     mybir.ActivationFunctionType.Exp)
                ews = []
                for tl in range(TB):
                    Ew = Epool.tile([P, WCOLS], fmm, name="Ew")
                    nc.vector.tensor_scalar(
                        Ew[:], iota[:, 0:WCOLS], d_st[:, tl:tl + 1],
                        ecols[:, tl:tl + 1],
                        op0=mybir.AluOpType.is_equal,
                        op1=mybir.AluOpType.mult)
                    ews.append(Ew)
                pending.append((b, a_st, ews))
                if len(pending) > int(_os.environ.get("FL_PIPE", "1")):
                    emit_emm(pending.pop(0))
            for pb in pending:
                emit_emm(pb)

        # ---- merge windows into global accumulators
        with ExitStack() as tail:
            acc2p = tail.enter_context(tc.tile_pool(name="acc2", bufs=1, space="PSUM"))
            acc2 = [acc2p.tile([P, Dp], f32, name=f"acc2_{i}")
                    for i in range(n_acc)]
            first = {a: True for a in range(n_acc)}
            last_k = {}
            for k, (r, a) in enumerate(merge_pairs):
                last_k[a] = k
            import os as _os
            if _os.environ.get("FL_EMM", "1") == "1" and _os.environ.get("FL_MERGE", "1") == "1":
                for k, (r, a) in enumerate(merge_pairs):
                    nc.tensor.matmul(acc2[a][:],
                                     lhsT=msb[:, k * P:(k + 1) * P],
                                     rhs=flush[:, r * Dp:(r + 1) * Dp],
                                     start=first[a], stop=(last_k[a] == k))
                    first[a] = False
            else:
                for a in range(n_acc):
                    nc.tensor.matmul(acc2[a][:], lhsT=ones1[:],
                                     rhs=flush[0:1, 0:Dp],
                                     start=True, stop=True)

            spool = tail.enter_context(tc.tile_pool(name="small", bufs=3 * n_acc))
            tpp = tail.enter_context(tc.tile_pool(name="tp", bufs=2, space="PSUM"))
            ptsb = [consts.tile([P, n_acc * P], f32, name=f"ptsb{h}")
                    for h in range(2)]
            for a in range(n_acc):
                dsafe = spool.tile([P, 1], f32)
                nc.vector.tensor_scalar_max(dsafe[:], acc2[a][:, D:D + 1], 1e-30)
                recip = spool.tile([P, 1], f32)
                nc.vector.reciprocal(recip[:], dsafe[:])
                pooledN = spool.tile([P, D], f32)
                nc.vector.tensor_scalar_mul(pooledN[:], acc2[a][:, 0:D], recip[:])
                for h in range(2):
                    tp = tpp.tile([P, P], f32)
                    nc.tensor.transpose(tp[:], pooledN[:, h * P:(h + 1) * P],
                                        ident[:])
                    nc.scalar.copy(ptsb[h][:, a * P:(a + 1) * P], tp[:])
            opp = tail.enter_context(tc.tile_pool(name="outp", bufs=2, space="PSUM"))
            opool = tail.enter_context(tc.tile_pool(name="osb", bufs=2))
            for a in range(n_acc):
                outp = opp.tile([P, D], f32)
                for h in range(2):
                    nc.tensor.matmul(outp[:], lhsT=ptsb[h][:, a * P:(a + 1) * P],
                                     rhs=wo_sb[:, h * D:(h + 1) * D],
                                     start=(h == 0), stop=False)
                nc.tensor.matmul(outp[:], lhsT=ones1[:], rhs=bo_sb[:],
                                 start=False, stop=True)
                outsb = opool.tile([P, D], f32)
                nc.vector.tensor_copy(outsb[:], outp[:])
                nc.sync.dma_start(out_d[a * P:(a + 1) * P, :], outsb[:])
    nc.compile()
    return nc


def kernel(atom_features, index_list, Wa, ba, Wo, bo):
    atom_features = np.asarray(atom_features, dtype=np.float32)
    index_list = np.asarray(index_list)
    n_atoms, D = atom_features.shape
    pl = plan(index_list, n_atoms)
    in_maps = build_core_inputs(pl, atom_features, Wa, Wo, bo)
    nc = build_program(pl, D=D, use_f32r=USE_F32R)

    from concourse.bass_utils import run_bass_kernel_spmd
    res = run_bass_kernel_spmd(nc, in_maps, list(range(NCORES)))
    outs = []
    for c, cc in enumerate(pl["cores"]):
        outs.append(res.results[c]["out"][:cc["n_seg"]])
    return np.concatenate(outs, axis=0)
